# revision 37
# baseline (speedup 1.0000x reference)
"""Multi-head attention (B=2, S=2048, D=1024, H=16, hd=64) on 8 NeuronCores.

Sharding: core c -> batch b=c//4, head-group hg=c%4 (4 heads each).
Per-core pipeline (bf16 matmuls, f32 PSUM):
  qkT proj (DVE bias-add) -> v proj (ones col via bv bias)
  -> attention per (query-pair qp, head-pair hp, query-half qh):
       scores: the two heads' K=64 matmuls write one psum tile and run
       concurrently in the PE array via row tiling (head A rows 0-63,
       head B rows 64-127); the single scores stream is triple-buffered
       so the PE never stalls on the exp engines.
       exp alternates by kb: even kb on ACT (table exp, bf16 out), odd
       kb on DVE (Schraudolph bit-trick: z = s*128/ln2 + MAGIC in f32;
       the low 16 bits of each z word are the bf16 bits of exp(s); the
       PV matmul reads them through a stride-2 bitcast view).
       PV matmuls accumulate [v | ones] so row 64 collects the softmax
       denominator for free.
       finisher: ACT Ln of the denom row, K=1 broadcast matmuls
       (col-tiled so head B lands on partitions 64-127), one ACT
       Exp(-x) -> 1/denom broadcast, DVE normalize multiplies.
  -> out-proj partials per qp (PSUM->SBUF copies split ACT/DVE).
Host gathers: sum head-group partials per batch, transpose, add bout.
"""
import sys
import types

import numpy as np
from contextlib import ExitStack

D = 1024
S = 2048
B = 2
HPC = 4          # heads per core
HD = 64          # head dim
NCORES = 8
QT = 512         # query tile (free dim of scores matmul)
KB = 128         # key block (partition dim of scores out)
NKB = S // KB    # 16
VW = HD + 1      # v width incl. ones column = 65
VROW = HPC * VW  # 260

# Schraudolph exp in bf16-bits domain, computed in f32:
#   z = s * (128/ln2) + (1.5*2^23 + 127*128 - c)   [c=6 centers rel err]
# bits(z) = 0x4B40_0000 | v16 where v16 = round(s*128/ln2 + 16250) are
# exactly the bf16 bits of ~exp(s) for |s| < 80. Scores here are |s|<3.
EXP_A = 184.6649652337873        # 128 / ln(2)
EXP_B = 12599162.0               # 12582912 + 127*128 - 6

_CACHE = {}


def _split_sync_waits(bir):
    """Walrus CoreV2/V3 codegen rejects >1 sync wait on one instruction
    ('Too many sync wait commands'). Hoist excess waits onto ENGINE_NOPs
    injected just before the offender in the same engine stream."""
    n = 0
    for fn in bir["functions"]:
        for blk in fn["blocks"]:
            out = []
            for inst in blk["instructions"]:
                si = inst.get("sync_info")
                ow = (si or {}).get("on_wait") or []
                if si is not None and len(ow) > 1:
                    for w in ow[:-1]:
                        n += 1
                        out.append({
                            "debug": inst.get("debug", 0),
                            "engine": inst["engine"],
                            "ins": [],
                            "name": f"I-ws{n}",
                            "opcode": "EventSemaphore",
                            "outs": [],
                            "sync_info": {"on_wait": [w], "on_update": []},
                        })
                    si["on_wait"] = [ow[-1]]
                out.append(inst)
            blk["instructions"] = out
    return bir


def _install_support():
    import json

    import concourse.bass as bass_mod
    import concourse.tile as tile_mod

    if not getattr(bass_mod.Bass, "_waitsplit_patched", False):
        orig = bass_mod.Bass.to_json_bytes

        def to_json_bytes(self, *a, **kw):
            data = json.loads(orig(self, *a, **kw))
            _split_sync_waits(data)
            return json.dumps(data).encode()

        bass_mod.Bass.to_json_bytes = to_json_bytes
        bass_mod.Bass._waitsplit_patched = True
    if not getattr(tile_mod.TileContext, "_drain_patched", False):
        import bass_rust
        ScopedClock = tile_mod.ScopedClock

        def _drain_and_barrier(self, tick_clock, wait_clock):
            drain_inst = self.nc.sync.drain()
            wait_clock.add_sem_waits(
                drain_inst.ins, ScopedClock({None: tick_clock.global_clock})
            )
            si = drain_inst.ins.sync_info
            if si is not None and len(si.on_wait) > 1:
                waits = list(si.on_wait)
                drain_inst.ins.sync_info = bass_rust.SyncInfo(
                    on_wait=waits[:1], on_update=list(si.on_update)
                )
                for w in waits[1:]:
                    extra = self.nc.sync.drain()
                    extra.ins.sync_info = bass_rust.SyncInfo(on_wait=[w], on_update=[])
            self.nc.all_engine_barrier()
            assert self.sems is not None
            popped = self.nc._tile_sem_poison_stack.pop()
            assert popped is self._sem_poison
            self.nc.clear_and_free_semaphores(list(self.sems.allocated().values()))
            self.nc.all_engine_barrier()

        tile_mod.TileContext._drain_and_barrier = _drain_and_barrier
        tile_mod.TileContext._drain_patched = True

    try:
        import antenv
        if "antenv.axon_hooks" not in sys.modules:
            mod = types.ModuleType("antenv.axon_hooks")
            mod._hook = None

            def set_axon_ntff_profile_hook(h, _mod=mod):
                _mod._hook = h

            def get_axon_ntff_profile_hook(_mod=mod):
                return _mod._hook

            mod.set_axon_ntff_profile_hook = set_axon_ntff_profile_hook
            mod.get_axon_ntff_profile_hook = get_axon_ntff_profile_hook
            sys.modules["antenv.axon_hooks"] = mod
            antenv.axon_hooks = mod
        from trn_agent_boot.trn_boot import _ntff_profile_via_ctypes
        hook = _ntff_profile_via_ctypes("/opt/axon/libaxon_pjrt.so")
        sys.modules["antenv.axon_hooks"].set_axon_ntff_profile_hook(hook)
        import concourse.bass_utils as bass_utils
        bass_utils.upload_artifacts = lambda d: d
    except Exception:
        pass


def _build_nc():
    import concourse.bass as bass
    import concourse.tile as tile
    from concourse import mybir

    f32 = mybir.dt.float32
    bf16 = mybir.dt.bfloat16
    AF = mybir.ActivationFunctionType
    MULT = mybir.AluOpType.mult
    ADD = mybir.AluOpType.add
    BYPASS = mybir.AluOpType.bypass

    nc = bass.Bass("TRN2", target_bir_lowering=False, debug=False,
                   num_devices=NCORES)
    xT_d = nc.dram_tensor("xT", [D, S], bf16, kind="ExternalInput").ap()
    wqk_d = nc.dram_tensor("wqk", [D, 512], bf16, kind="ExternalInput").ap()
    bqk_d = nc.dram_tensor("bqk", [128, 4], f32, kind="ExternalInput").ap()
    wv_d = nc.dram_tensor("wv", [D, VROW], bf16, kind="ExternalInput").ap()
    bv_d = nc.dram_tensor("bv", [128, VROW], f32, kind="ExternalInput").ap()
    wo_d = nc.dram_tensor("wo", [256, D], bf16, kind="ExternalInput").ap()
    out_d = nc.dram_tensor("out", [D, S], f32, kind="ExternalOutput").ap()

    def zview(zt):
        # bf16 exp values = low halfword of each magic f32
        return zt[:].bitcast(bf16).rearrange("p (n two) -> p n two", two=2)[:, :, 0]

    with tile.TileContext(nc) as tc, ExitStack() as ctx:
        persist = ctx.enter_context(tc.tile_pool(name="persist", bufs=1))
        # PSUM budget (8 banks): ps 2x[128,1024]=4 banks, pvp 2x[65,1024]=4.
        # Each scores tile has a single exp consumer (ACT or DVE), so
        # two buffers keep the PE a full kb ahead; double-buffered pv
        # lets the next (hp,qh) iteration start accumulating while the
        # previous finisher chain is still draining.
        ps = ctx.enter_context(
            tc.tile_pool(name="ps", bufs=2, space=bass.MemorySpace.PSUM))
        pvp = ctx.enter_context(
            tc.tile_pool(name="pvp", bufs=2, space=bass.MemorySpace.PSUM))
        expp = ctx.enter_context(tc.tile_pool(name="expp", bufs=3))
        zp = ctx.enter_context(tc.tile_pool(name="zp", bufs=3))
        lnp = ctx.enter_context(tc.tile_pool(name="lnp", bufs=2))
        rec64p = ctx.enter_context(tc.tile_pool(name="rec64p", bufs=2))
        outp = ctx.enter_context(tc.tile_pool(name="outp", bufs=2))

        xT_sb = persist.tile([128, 8 * S], bf16)
        wqk_sb = persist.tile([128, 8 * 512], bf16)
        bqk_sb = persist.tile([128, 4], f32)
        wv_sb = persist.tile([128, 8 * VROW], bf16)
        bv_sb = persist.tile([128, VROW], f32)
        wo_sb = persist.tile([128, 2 * D], bf16)
        qkT_sb = persist.tile([128, 4 * S], bf16)
        v_sb = persist.tile([128, NKB * VROW], bf16)
        outT_sb = persist.tile([128, 2 * S], bf16)
        ones64 = persist.tile([1, 64], f32)
        nc.vector.memset(ones64[:], 1.0)

        for kb in range(8):
            nc.sync.dma_start(xT_sb[:, kb * S:(kb + 1) * S],
                              xT_d[kb * 128:(kb + 1) * 128, :])
            nc.sync.dma_start(wqk_sb[:, kb * 512:(kb + 1) * 512],
                              wqk_d[kb * 128:(kb + 1) * 128, :])
            nc.sync.dma_start(wv_sb[:, kb * VROW:(kb + 1) * VROW],
                              wv_d[kb * 128:(kb + 1) * 128, :])
        nc.sync.dma_start(bqk_sb[:], bqk_d[:])
        nc.sync.dma_start(bv_sb[:], bv_d[:])
        for cb in range(2):
            nc.sync.dma_start(wo_sb[:, cb * D:(cb + 1) * D],
                              wo_d[cb * 128:(cb + 1) * 128, :])

        # qkT projection: m-tiles m0=[q_h0|q_h1] m1=[q_h2|q_h3] m2=[k_h0|k_h1]
        # m3=[k_h2|k_h3]; bias-add on DVE (per-partition scalar).
        for m in range(4):
            for npair in range(2):
                ss = ps.tile([128, 2 * QT], f32, tag="s")
                for kb in range(8):
                    for half in range(2):
                        n = npair * 2 + half
                        nc.tensor.matmul(
                            ss[:, half * QT:(half + 1) * QT],
                            wqk_sb[:, kb * 512 + m * 128: kb * 512 + (m + 1) * 128],
                            xT_sb[:, kb * S + n * QT: kb * S + (n + 1) * QT],
                            start=(kb == 0), stop=(kb == 7),
                            skip_group_check=True)
                nc.vector.tensor_scalar(
                    qkT_sb[:, m * S + npair * 2 * QT: m * S + (npair + 1) * 2 * QT],
                    ss[:], bqk_sb[:, m:m + 1], None, ADD, BYPASS)

        # v projection (+ ones column via bv): v_sb[128, 16*260]
        for sb in range(NKB):
            ss = ps.tile([128, 2 * QT], f32, tag="s")
            for kb in range(8):
                nc.tensor.matmul(
                    ss[:, 0:VROW],
                    xT_sb[:, kb * S + sb * 128: kb * S + (sb + 1) * 128],
                    wv_sb[:, kb * VROW:(kb + 1) * VROW],
                    start=(kb == 0), stop=(kb == 7))
            nc.vector.tensor_add(
                v_sb[:, sb * VROW:(sb + 1) * VROW], ss[:, 0:VROW], bv_sb[:])

        # attention per (query pair qp, head pair hp, query half qh).
        # Each kb produces ONE psum tile holding [head-A | head-B]
        # scores for this query half: the two matmuls into one tile
        # stay adjacent in the PE stream and their disjoint row groups
        # (A rows 0-63, B rows 64-127) run concurrently. The single
        # scores stream is triple-buffered (ps bufs=3) so the PE never
        # waits on the exp engines; exp alternates ACT/DVE by kb.
        def make_finisher(pv2, hp, qp, qh):
            # pv2 [65, 2*QT] = [A | B] for query half qh; row 64 holds
            # the softmax denominators. Deferred past the next
            # iteration's first score tiles. The two ACT reads (Ln +
            # copy to SBUF) release pv2 early; the rest of the
            # reciprocal chain works off the copy, off the PE critical
            # path.
            def emit():
                ocol = hp * S + qp * 2 * QT + qh * QT
                lnrow = lnp.tile([1, 2 * QT], f32, tag="ln")
                nc.scalar.activation(lnrow[:], pv2[64:65, :], AF.Ln)
                ps_rec = ps.tile([128, 2 * QT], f32, tag="s")
                for j in range(2):
                    pr = j * 64
                    nc.tensor.matmul(
                        ps_rec[pr:pr + 64, 0:QT],
                        ones64[:], lnrow[:, j * QT:(j + 1) * QT],
                        start=True, stop=True,
                        tile_position=(0, pr), skip_group_check=True)
                rec64 = rec64p.tile([128, QT], f32, tag="r64")
                nc.scalar.activation(rec64[:], ps_rec[:, 0:QT], AF.Exp,
                                     scale=-1.0)
                for j in range(2):
                    pr = j * 64
                    nc.vector.tensor_tensor(
                        out=outT_sb[pr:pr + 64, ocol: ocol + QT],
                        in0=pv2[0:64, j * QT:(j + 1) * QT],
                        in1=rec64[pr:pr + 64, :],
                        op=MULT)
            return emit

        pending = None
        for qp in range(2):
            for hp in range(2):
                kbase = (2 + hp) * S
                hA, hB = 2 * hp, 2 * hp + 1
                for qh in range(2):
                    qs = hp * S + qp * 2 * QT + qh * QT
                    pv2 = pvp.tile([VW, 2 * QT], f32, tag="pv2", name="pv2")
                    prev = None  # rhs of kb-1
                    for kb in range(NKB):
                        kA = qkT_sb[0:64,
                                    kbase + kb * 128: kbase + (kb + 1) * 128]
                        kB = qkT_sb[64:128,
                                    kbase + kb * 128: kbase + (kb + 1) * 128]
                        s = ps.tile([128, 2 * QT], f32, tag="s", name="s")
                        nc.tensor.matmul(
                            s[:, 0:QT], kA, qkT_sb[0:64, qs: qs + QT],
                            start=True, stop=True, skip_group_check=True)
                        nc.tensor.matmul(
                            s[:, QT:2 * QT], kB, qkT_sb[64:128, qs: qs + QT],
                            start=True, stop=True, skip_group_check=True)
                        if kb % 2 == 0:
                            e0 = expp.tile([128, 2 * QT], bf16, tag="e")
                            nc.scalar.activation(e0[:], s[:], AF.Exp)
                            rhs = e0[:]
                        else:
                            z1 = zp.tile([128, 2 * QT], f32, tag="z")
                            nc.vector.tensor_scalar(z1[:], s[:],
                                                    EXP_A, EXP_B, MULT, ADD)
                            rhs = zview(z1)
                        if pending is not None:
                            pending()
                            pending = None
                        if prev is not None:
                            emit_pv(nc, pv2, v_sb, hA, hB, *prev)
                        prev = (rhs, kb)
                    emit_pv(nc, pv2, v_sb, hA, hB, *prev, last=True)
                    pending = make_finisher(pv2, hp, qp, qh)

            pending()
            pending = None

            # out projection for this query pair (both tiles at once)
            for dt in range(8):
                ss = ps.tile([128, 2 * QT], f32, tag="s")
                for half in range(2):
                    qi = qp * 2 + half
                    for cb in range(2):
                        nc.tensor.matmul(
                            ss[:, half * QT:(half + 1) * QT],
                            wo_sb[:, cb * D + dt * 128: cb * D + (dt + 1) * 128],
                            outT_sb[:, cb * S + qi * QT: cb * S + (qi + 1) * QT],
                            start=(cb == 0), stop=(cb == 1),
                            skip_group_check=True)
                osb = outp.tile([128, 2 * QT], f32, tag="ob")
                if dt % 2 == 0:
                    nc.scalar.activation(osb[:], ss[:], AF.Identity)
                else:
                    nc.vector.tensor_copy(osb[:], ss[:])
                nc.sync.dma_start(
                    out_d[dt * 128:(dt + 1) * 128, qp * 2 * QT:(qp + 1) * 2 * QT],
                    osb[:])
    return nc


def emit_pv(nc, pv2, v_sb, hA, hB, rhs, kb, last=False):
    # pv2 layout: [A | B], each QT wide; rhs = exp tile [128, 2*QT]
    # with head A in the left half and head B in the right half.
    vA = v_sb[:, kb * VROW + hA * VW: kb * VROW + (hA + 1) * VW]
    vB = v_sb[:, kb * VROW + hB * VW: kb * VROW + (hB + 1) * VW]
    start = (kb == 0)
    stop = last
    nc.tensor.matmul(pv2[:, 0:QT], vA, rhs[:, 0:QT],
                     start=start, stop=stop, skip_group_check=True)
    nc.tensor.matmul(pv2[:, QT:2 * QT], vB, rhs[:, QT:2 * QT],
                     start=start, stop=stop, skip_group_check=True)


def _get_nc():
    if "nc" not in _CACHE:
        _install_support()
        _CACHE["nc"] = _build_nc()
    return _CACHE["nc"]


LAST_EXEC_NS = None


def kernel(x, Wqkv, bqkv, Wout, bout):
    from ml_dtypes import bfloat16
    from concourse.bass_utils import run_bass_kernel_spmd

    nc = _get_nc()

    x = np.asarray(x, np.float32)
    Wqkv = np.asarray(Wqkv, np.float32)
    bqkv = np.asarray(bqkv, np.float32)
    Wout = np.asarray(Wout, np.float32)
    bout = np.asarray(bout, np.float32)

    xT = [np.ascontiguousarray(x[b].T).astype(bfloat16) for b in range(B)]

    in_maps = []
    for c in range(NCORES):
        b, hg = divmod(c, HPC)
        heads = [hg * HPC + j for j in range(HPC)]

        wqk = np.empty((D, 512), np.float32)
        bqk = np.empty(512, np.float32)
        for j, h in enumerate(heads):
            base = h * 192
            wqk[:, j * 64:(j + 1) * 64] = Wqkv[:, base:base + 64] * 0.125
            wqk[:, 256 + j * 64: 256 + (j + 1) * 64] = Wqkv[:, base + 64:base + 128]
            bqk[j * 64:(j + 1) * 64] = bqkv[base:base + 64] * 0.125
            bqk[256 + j * 64: 256 + (j + 1) * 64] = bqkv[base + 64:base + 128]
        bqk = np.ascontiguousarray(bqk.reshape(4, 128).T)

        wv = np.zeros((D, VROW), np.float32)
        bv_row = np.zeros(VROW, np.float32)
        for j, h in enumerate(heads):
            base = h * 192 + 128
            wv[:, j * VW: j * VW + 64] = Wqkv[:, base:base + 64]
            bv_row[j * VW: j * VW + 64] = bqkv[base:base + 64]
            bv_row[j * VW + 64] = 1.0
        bv = np.broadcast_to(bv_row, (128, VROW)).copy()

        wo = np.ascontiguousarray(Wout[hg * 256:(hg + 1) * 256, :])

        in_maps.append({
            "xT": xT[b],
            "wqk": wqk.astype(bfloat16),
            "bqk": bqk,
            "wv": wv.astype(bfloat16),
            "bv": bv,
            "wo": wo.astype(bfloat16),
        })

    res = run_bass_kernel_spmd(nc, in_maps, core_ids=list(range(NCORES)))
    global LAST_EXEC_NS
    LAST_EXEC_NS = getattr(res, "exec_time_ns", None)

    out = np.empty((B, S, D), np.float32)
    for b in range(B):
        acc = res.results[b * HPC + 0]["out"].astype(np.float32)
        for hg in range(1, HPC):
            acc = acc + res.results[b * HPC + hg]["out"].astype(np.float32)
        out[b] = acc.T + bout
    return out


# revision 38
# speedup vs baseline: 1.1032x; 1.1032x over previous
"""Multi-head attention (B=2, S=2048, D=1024, H=16, hd=64) on 8 NeuronCores.

Sharding: core c -> batch b=c//4, head-group hg=c%4 (4 heads each).
Per-core pipeline (bf16 matmuls, f32 PSUM):
  qkT proj (DVE bias-add) -> v proj (ones col via bv bias)
  -> attention per (query-pair qp, head-pair hp, query-half qh):
       scores: the two heads' K=64 matmuls write one psum tile and run
       concurrently in the PE array via row tiling (head A rows 0-63,
       head B rows 64-127); the single scores stream is triple-buffered
       so the PE never stalls on the exp engines.
       exp alternates by kb: even kb on ACT (table exp, bf16 out), odd
       kb on DVE (Schraudolph bit-trick: z = s*128/ln2 + MAGIC in f32;
       the low 16 bits of each z word are the bf16 bits of exp(s); the
       PV matmul reads them through a stride-2 bitcast view).
       PV matmuls accumulate [v | ones] so row 64 collects the softmax
       denominator for free.
       finisher: ACT Ln of the denom row, K=1 broadcast matmuls
       (col-tiled so head B lands on partitions 64-127), one ACT
       Exp(-x) -> 1/denom broadcast, DVE normalize multiplies.
  -> out-proj partials per qp (PSUM->SBUF copies split ACT/DVE).
Host gathers: sum head-group partials per batch, transpose, add bout.
"""
import sys
import types

import numpy as np
from contextlib import ExitStack

D = 1024
S = 2048
B = 2
HPC = 4          # heads per core
HD = 64          # head dim
NCORES = 8
QT = 512         # query tile (free dim of scores matmul)
KB = 128         # key block (partition dim of scores out)
NKB = S // KB    # 16
VW = HD + 1      # v width incl. ones column = 65
VROW = HPC * VW  # 260

# Schraudolph exp in bf16-bits domain, computed in f32:
#   z = s * (128/ln2) + (1.5*2^23 + 127*128 - c)   [c=6 centers rel err]
# bits(z) = 0x4B40_0000 | v16 where v16 = round(s*128/ln2 + 16250) are
# exactly the bf16 bits of ~exp(s) for |s| < 80. Scores here are |s|<3.
EXP_A = 184.6649652337873        # 128 / ln(2)
EXP_B = 12599162.0               # 12582912 + 127*128 - 6

_CACHE = {}


def _split_sync_waits(bir):
    """Walrus CoreV2/V3 codegen rejects >1 sync wait on one instruction
    ('Too many sync wait commands'). Hoist excess waits onto ENGINE_NOPs
    injected just before the offender in the same engine stream."""
    n = 0
    for fn in bir["functions"]:
        for blk in fn["blocks"]:
            out = []
            for inst in blk["instructions"]:
                si = inst.get("sync_info")
                ow = (si or {}).get("on_wait") or []
                if si is not None and len(ow) > 1:
                    for w in ow[:-1]:
                        n += 1
                        out.append({
                            "debug": inst.get("debug", 0),
                            "engine": inst["engine"],
                            "ins": [],
                            "name": f"I-ws{n}",
                            "opcode": "EventSemaphore",
                            "outs": [],
                            "sync_info": {"on_wait": [w], "on_update": []},
                        })
                    si["on_wait"] = [ow[-1]]
                out.append(inst)
            blk["instructions"] = out
    return bir


def _install_support():
    import json

    import concourse.bass as bass_mod
    import concourse.tile as tile_mod

    if not getattr(bass_mod.Bass, "_waitsplit_patched", False):
        orig = bass_mod.Bass.to_json_bytes

        def to_json_bytes(self, *a, **kw):
            data = json.loads(orig(self, *a, **kw))
            _split_sync_waits(data)
            return json.dumps(data).encode()

        bass_mod.Bass.to_json_bytes = to_json_bytes
        bass_mod.Bass._waitsplit_patched = True
    if not getattr(tile_mod.TileContext, "_drain_patched", False):
        import bass_rust
        ScopedClock = tile_mod.ScopedClock

        def _drain_and_barrier(self, tick_clock, wait_clock):
            drain_inst = self.nc.sync.drain()
            wait_clock.add_sem_waits(
                drain_inst.ins, ScopedClock({None: tick_clock.global_clock})
            )
            si = drain_inst.ins.sync_info
            if si is not None and len(si.on_wait) > 1:
                waits = list(si.on_wait)
                drain_inst.ins.sync_info = bass_rust.SyncInfo(
                    on_wait=waits[:1], on_update=list(si.on_update)
                )
                for w in waits[1:]:
                    extra = self.nc.sync.drain()
                    extra.ins.sync_info = bass_rust.SyncInfo(on_wait=[w], on_update=[])
            self.nc.all_engine_barrier()
            assert self.sems is not None
            popped = self.nc._tile_sem_poison_stack.pop()
            assert popped is self._sem_poison
            self.nc.clear_and_free_semaphores(list(self.sems.allocated().values()))
            self.nc.all_engine_barrier()

        tile_mod.TileContext._drain_and_barrier = _drain_and_barrier
        tile_mod.TileContext._drain_patched = True

    try:
        import antenv
        if "antenv.axon_hooks" not in sys.modules:
            mod = types.ModuleType("antenv.axon_hooks")
            mod._hook = None

            def set_axon_ntff_profile_hook(h, _mod=mod):
                _mod._hook = h

            def get_axon_ntff_profile_hook(_mod=mod):
                return _mod._hook

            mod.set_axon_ntff_profile_hook = set_axon_ntff_profile_hook
            mod.get_axon_ntff_profile_hook = get_axon_ntff_profile_hook
            sys.modules["antenv.axon_hooks"] = mod
            antenv.axon_hooks = mod
        from trn_agent_boot.trn_boot import _ntff_profile_via_ctypes
        hook = _ntff_profile_via_ctypes("/opt/axon/libaxon_pjrt.so")
        sys.modules["antenv.axon_hooks"].set_axon_ntff_profile_hook(hook)
        import concourse.bass_utils as bass_utils
        bass_utils.upload_artifacts = lambda d: d
    except Exception:
        pass


def _build_nc():
    import concourse.bass as bass
    import concourse.tile as tile
    from concourse import mybir

    f32 = mybir.dt.float32
    bf16 = mybir.dt.bfloat16
    AF = mybir.ActivationFunctionType
    MULT = mybir.AluOpType.mult
    ADD = mybir.AluOpType.add
    BYPASS = mybir.AluOpType.bypass

    nc = bass.Bass("TRN2", target_bir_lowering=False, debug=False,
                   num_devices=NCORES)
    xT_d = nc.dram_tensor("xT", [D, S], bf16, kind="ExternalInput").ap()
    wqk_d = nc.dram_tensor("wqk", [D, 512], bf16, kind="ExternalInput").ap()
    bqk_d = nc.dram_tensor("bqk", [128, 4], f32, kind="ExternalInput").ap()
    wv_d = nc.dram_tensor("wv", [D, VROW], bf16, kind="ExternalInput").ap()
    bv_d = nc.dram_tensor("bv", [128, VROW], f32, kind="ExternalInput").ap()
    wo_d = nc.dram_tensor("wo", [256, D], bf16, kind="ExternalInput").ap()
    out_d = nc.dram_tensor("out", [D, S], f32, kind="ExternalOutput").ap()

    def zview(zt):
        # bf16 exp values = low halfword of each magic f32
        return zt[:].bitcast(bf16).rearrange("p (n two) -> p n two", two=2)[:, :, 0]

    with tile.TileContext(nc) as tc, ExitStack() as ctx:
        persist = ctx.enter_context(tc.tile_pool(name="persist", bufs=1))
        # PSUM budget (8 banks): ps 3x[128,1024]=6 banks, pvp [65,1024]=2.
        ps = ctx.enter_context(
            tc.tile_pool(name="ps", bufs=3, space=bass.MemorySpace.PSUM))
        pvp = ctx.enter_context(
            tc.tile_pool(name="pvp", bufs=1, space=bass.MemorySpace.PSUM))
        expp = ctx.enter_context(tc.tile_pool(name="expp", bufs=3))
        zp = ctx.enter_context(tc.tile_pool(name="zp", bufs=3))
        lnp = ctx.enter_context(tc.tile_pool(name="lnp", bufs=2))
        rec64p = ctx.enter_context(tc.tile_pool(name="rec64p", bufs=2))
        outp = ctx.enter_context(tc.tile_pool(name="outp", bufs=2))

        xT_sb = persist.tile([128, 8 * S], bf16)
        wqk_sb = persist.tile([128, 8 * 512], bf16)
        bqk_sb = persist.tile([128, 4], f32)
        wv_sb = persist.tile([128, 8 * VROW], bf16)
        bv_sb = persist.tile([128, VROW], f32)
        wo_sb = persist.tile([128, 2 * D], bf16)
        qkT_sb = persist.tile([128, 4 * S], bf16)
        v_sb = persist.tile([128, NKB * VROW], bf16)
        outT_sb = persist.tile([128, 2 * S], bf16)
        ones64 = persist.tile([1, 64], f32)
        nc.vector.memset(ones64[:], 1.0)

        for kb in range(8):
            nc.sync.dma_start(xT_sb[:, kb * S:(kb + 1) * S],
                              xT_d[kb * 128:(kb + 1) * 128, :])
            nc.sync.dma_start(wqk_sb[:, kb * 512:(kb + 1) * 512],
                              wqk_d[kb * 128:(kb + 1) * 128, :])
            nc.sync.dma_start(wv_sb[:, kb * VROW:(kb + 1) * VROW],
                              wv_d[kb * 128:(kb + 1) * 128, :])
        nc.sync.dma_start(bqk_sb[:], bqk_d[:])
        nc.sync.dma_start(bv_sb[:], bv_d[:])
        for cb in range(2):
            nc.sync.dma_start(wo_sb[:, cb * D:(cb + 1) * D],
                              wo_d[cb * 128:(cb + 1) * 128, :])

        # qkT projection: m-tiles m0=[q_h0|q_h1] m1=[q_h2|q_h3] m2=[k_h0|k_h1]
        # m3=[k_h2|k_h3]; bias-add on DVE (per-partition scalar).
        for m in range(4):
            for npair in range(2):
                ss = ps.tile([128, 2 * QT], f32, tag="s")
                for kb in range(8):
                    for half in range(2):
                        n = npair * 2 + half
                        nc.tensor.matmul(
                            ss[:, half * QT:(half + 1) * QT],
                            wqk_sb[:, kb * 512 + m * 128: kb * 512 + (m + 1) * 128],
                            xT_sb[:, kb * S + n * QT: kb * S + (n + 1) * QT],
                            start=(kb == 0), stop=(kb == 7),
                            skip_group_check=True)
                nc.vector.tensor_scalar(
                    qkT_sb[:, m * S + npair * 2 * QT: m * S + (npair + 1) * 2 * QT],
                    ss[:], bqk_sb[:, m:m + 1], None, ADD, BYPASS)

        # v projection (+ ones column via bv): v_sb[128, 16*260]
        for sb in range(NKB):
            ss = ps.tile([128, 2 * QT], f32, tag="s")
            for kb in range(8):
                nc.tensor.matmul(
                    ss[:, 0:VROW],
                    xT_sb[:, kb * S + sb * 128: kb * S + (sb + 1) * 128],
                    wv_sb[:, kb * VROW:(kb + 1) * VROW],
                    start=(kb == 0), stop=(kb == 7))
            nc.vector.tensor_add(
                v_sb[:, sb * VROW:(sb + 1) * VROW], ss[:, 0:VROW], bv_sb[:])

        # attention per (query pair qp, head pair hp, query half qh).
        # Each kb produces ONE psum tile holding [head-A | head-B]
        # scores for this query half: the two matmuls into one tile
        # stay adjacent in the PE stream and their disjoint row groups
        # (A rows 0-63, B rows 64-127) run concurrently. The single
        # scores stream is triple-buffered (ps bufs=3) so the PE never
        # waits on the exp engines; exp alternates ACT/DVE by kb.
        def make_finisher(pv2, hp, qp, qh):
            # pv2 [65, 2*QT] = [A | B] for query half qh; row 64 holds
            # the softmax denominators. Deferred past the next
            # iteration's first score tiles. The two ACT reads (Ln +
            # copy to SBUF) release pv2 early; the rest of the
            # reciprocal chain works off the copy, off the PE critical
            # path.
            def emit():
                ocol = hp * S + qp * 2 * QT + qh * QT
                lnrow = lnp.tile([1, 2 * QT], f32, tag="ln")
                nc.scalar.activation(lnrow[:], pv2[64:65, :], AF.Ln)
                ps_rec = ps.tile([128, 2 * QT], f32, tag="s")
                for j in range(2):
                    pr = j * 64
                    nc.tensor.matmul(
                        ps_rec[pr:pr + 64, 0:QT],
                        ones64[:], lnrow[:, j * QT:(j + 1) * QT],
                        start=True, stop=True,
                        tile_position=(0, pr), skip_group_check=True)
                rec64 = rec64p.tile([128, QT], f32, tag="r64")
                nc.scalar.activation(rec64[:], ps_rec[:, 0:QT], AF.Exp,
                                     scale=-1.0)
                for j in range(2):
                    pr = j * 64
                    nc.vector.tensor_tensor(
                        out=outT_sb[pr:pr + 64, ocol: ocol + QT],
                        in0=pv2[0:64, j * QT:(j + 1) * QT],
                        in1=rec64[pr:pr + 64, :],
                        op=MULT)
            return emit

        pending = None
        for qp in range(2):
            for hp in range(2):
                kbase = (2 + hp) * S
                hA, hB = 2 * hp, 2 * hp + 1
                for qh in range(2):
                    qs = hp * S + qp * 2 * QT + qh * QT
                    pv2 = pvp.tile([VW, 2 * QT], f32, tag="pv2", name="pv2")
                    prev = None  # rhs of kb-1
                    for kb in range(NKB):
                        kA = qkT_sb[0:64,
                                    kbase + kb * 128: kbase + (kb + 1) * 128]
                        kB = qkT_sb[64:128,
                                    kbase + kb * 128: kbase + (kb + 1) * 128]
                        s = ps.tile([128, 2 * QT], f32, tag="s", name="s")
                        nc.tensor.matmul(
                            s[:, 0:QT], kA, qkT_sb[0:64, qs: qs + QT],
                            start=True, stop=True, skip_group_check=True)
                        nc.tensor.matmul(
                            s[:, QT:2 * QT], kB, qkT_sb[64:128, qs: qs + QT],
                            start=True, stop=True, skip_group_check=True)
                        if kb % 2 == 0:
                            e0 = expp.tile([128, 2 * QT], bf16, tag="e")
                            nc.scalar.activation(e0[:], s[:], AF.Exp)
                            rhs = e0[:]
                        else:
                            z1 = zp.tile([128, 2 * QT], f32, tag="z")
                            nc.vector.tensor_scalar(z1[:], s[:],
                                                    EXP_A, EXP_B, MULT, ADD)
                            rhs = zview(z1)
                        if pending is not None:
                            pending()
                            pending = None
                        if prev is not None:
                            emit_pv(nc, pv2, v_sb, hA, hB, *prev)
                        prev = (rhs, kb)
                    emit_pv(nc, pv2, v_sb, hA, hB, *prev, last=True)
                    pending = make_finisher(pv2, hp, qp, qh)

            pending()
            pending = None

            # out projection for this query pair (both tiles at once)
            for dt in range(8):
                ss = ps.tile([128, 2 * QT], f32, tag="s")
                for half in range(2):
                    qi = qp * 2 + half
                    for cb in range(2):
                        nc.tensor.matmul(
                            ss[:, half * QT:(half + 1) * QT],
                            wo_sb[:, cb * D + dt * 128: cb * D + (dt + 1) * 128],
                            outT_sb[:, cb * S + qi * QT: cb * S + (qi + 1) * QT],
                            start=(cb == 0), stop=(cb == 1),
                            skip_group_check=True)
                osb = outp.tile([128, 2 * QT], f32, tag="ob")
                if dt % 2 == 0:
                    nc.scalar.activation(osb[:], ss[:], AF.Identity)
                else:
                    nc.vector.tensor_copy(osb[:], ss[:])
                nc.sync.dma_start(
                    out_d[dt * 128:(dt + 1) * 128, qp * 2 * QT:(qp + 1) * 2 * QT],
                    osb[:])
    return nc


def emit_pv(nc, pv2, v_sb, hA, hB, rhs, kb, last=False):
    # pv2 layout: [A | B], each QT wide; rhs = exp tile [128, 2*QT]
    # with head A in the left half and head B in the right half.
    vA = v_sb[:, kb * VROW + hA * VW: kb * VROW + (hA + 1) * VW]
    vB = v_sb[:, kb * VROW + hB * VW: kb * VROW + (hB + 1) * VW]
    start = (kb == 0)
    stop = last
    nc.tensor.matmul(pv2[:, 0:QT], vA, rhs[:, 0:QT],
                     start=start, stop=stop, skip_group_check=True)
    nc.tensor.matmul(pv2[:, QT:2 * QT], vB, rhs[:, QT:2 * QT],
                     start=start, stop=stop, skip_group_check=True)


def _get_nc():
    if "nc" not in _CACHE:
        _install_support()
        _CACHE["nc"] = _build_nc()
    return _CACHE["nc"]


LAST_EXEC_NS = None


def kernel(x, Wqkv, bqkv, Wout, bout):
    from ml_dtypes import bfloat16
    from concourse.bass_utils import run_bass_kernel_spmd

    nc = _get_nc()

    x = np.asarray(x, np.float32)
    Wqkv = np.asarray(Wqkv, np.float32)
    bqkv = np.asarray(bqkv, np.float32)
    Wout = np.asarray(Wout, np.float32)
    bout = np.asarray(bout, np.float32)

    xT = [np.ascontiguousarray(x[b].T).astype(bfloat16) for b in range(B)]

    in_maps = []
    for c in range(NCORES):
        b, hg = divmod(c, HPC)
        heads = [hg * HPC + j for j in range(HPC)]

        wqk = np.empty((D, 512), np.float32)
        bqk = np.empty(512, np.float32)
        for j, h in enumerate(heads):
            base = h * 192
            wqk[:, j * 64:(j + 1) * 64] = Wqkv[:, base:base + 64] * 0.125
            wqk[:, 256 + j * 64: 256 + (j + 1) * 64] = Wqkv[:, base + 64:base + 128]
            bqk[j * 64:(j + 1) * 64] = bqkv[base:base + 64] * 0.125
            bqk[256 + j * 64: 256 + (j + 1) * 64] = bqkv[base + 64:base + 128]
        bqk = np.ascontiguousarray(bqk.reshape(4, 128).T)

        wv = np.zeros((D, VROW), np.float32)
        bv_row = np.zeros(VROW, np.float32)
        for j, h in enumerate(heads):
            base = h * 192 + 128
            wv[:, j * VW: j * VW + 64] = Wqkv[:, base:base + 64]
            bv_row[j * VW: j * VW + 64] = bqkv[base:base + 64]
            bv_row[j * VW + 64] = 1.0
        bv = np.broadcast_to(bv_row, (128, VROW)).copy()

        wo = np.ascontiguousarray(Wout[hg * 256:(hg + 1) * 256, :])

        in_maps.append({
            "xT": xT[b],
            "wqk": wqk.astype(bfloat16),
            "bqk": bqk,
            "wv": wv.astype(bfloat16),
            "bv": bv,
            "wo": wo.astype(bfloat16),
        })

    res = run_bass_kernel_spmd(nc, in_maps, core_ids=list(range(NCORES)))
    global LAST_EXEC_NS
    LAST_EXEC_NS = getattr(res, "exec_time_ns", None)

    out = np.empty((B, S, D), np.float32)
    for b in range(B):
        acc = res.results[b * HPC + 0]["out"].astype(np.float32)
        for hg in range(1, HPC):
            acc = acc + res.results[b * HPC + hg]["out"].astype(np.float32)
        out[b] = acc.T + bout
    return out


# revision 40
# speedup vs baseline: 1.1245x; 1.0193x over previous
"""Multi-head attention (B=2, S=2048, D=1024, H=16, hd=64) on 8 NeuronCores.

Sharding: core c -> batch b=c//4, head-group hg=c%4 (4 heads each).
Per-core pipeline (bf16 matmuls, f32 PSUM):
  qkT proj (DVE bias-add) -> v proj (ones col via bv bias)
  -> attention per (query-pair qp, head-pair hp, query-half qh):
       scores: the two heads' K=64 matmuls write one psum tile and run
       concurrently in the PE array via row tiling (head A rows 0-63,
       head B rows 64-127); the single scores stream is triple-buffered
       so the PE never stalls on the exp engines.
       exp alternates by kb: even kb on ACT (table exp, bf16 out), odd
       kb on DVE (Schraudolph bit-trick: z = s*128/ln2 + MAGIC in f32;
       the low 16 bits of each z word are the bf16 bits of exp(s); the
       PV matmul reads them through a stride-2 bitcast view).
       PV matmuls accumulate [v | ones] so row 64 collects the softmax
       denominator for free.
       finisher: ACT Ln of the denom row, K=1 broadcast matmuls
       (col-tiled so head B lands on partitions 64-127), one ACT
       Exp(-x) -> 1/denom broadcast, DVE normalize multiplies.
  -> out-proj partials per qp (PSUM->SBUF copies split ACT/DVE).
Host gathers: sum head-group partials per batch, transpose, add bout.
"""
import sys
import types

import numpy as np
from contextlib import ExitStack

D = 1024
S = 2048
B = 2
HPC = 4          # heads per core
HD = 64          # head dim
NCORES = 8
QT = 512         # query tile (free dim of scores matmul)
KB = 128         # key block (partition dim of scores out)
NKB = S // KB    # 16
VW = HD + 1      # v width incl. ones column = 65
VROW = HPC * VW  # 260

# Schraudolph exp in bf16-bits domain, computed in f32:
#   z = s * (128/ln2) + (1.5*2^23 + 127*128 - c)   [c=6 centers rel err]
# bits(z) = 0x4B40_0000 | v16 where v16 = round(s*128/ln2 + 16250) are
# exactly the bf16 bits of ~exp(s) for |s| < 80. Scores here are |s|<3.
EXP_A = 184.6649652337873        # 128 / ln(2)
EXP_B = 12599162.0               # 12582912 + 127*128 - 6

_CACHE = {}


def _split_sync_waits(bir):
    """Walrus CoreV2/V3 codegen rejects >1 sync wait on one instruction
    ('Too many sync wait commands'). Hoist excess waits onto ENGINE_NOPs
    injected just before the offender in the same engine stream."""
    n = 0
    for fn in bir["functions"]:
        for blk in fn["blocks"]:
            out = []
            for inst in blk["instructions"]:
                si = inst.get("sync_info")
                ow = (si or {}).get("on_wait") or []
                if si is not None and len(ow) > 1:
                    for w in ow[:-1]:
                        n += 1
                        out.append({
                            "debug": inst.get("debug", 0),
                            "engine": inst["engine"],
                            "ins": [],
                            "name": f"I-ws{n}",
                            "opcode": "EventSemaphore",
                            "outs": [],
                            "sync_info": {"on_wait": [w], "on_update": []},
                        })
                    si["on_wait"] = [ow[-1]]
                out.append(inst)
            blk["instructions"] = out
    return bir


def _install_support():
    import json

    import concourse.bass as bass_mod
    import concourse.tile as tile_mod

    if not getattr(bass_mod.Bass, "_waitsplit_patched", False):
        orig = bass_mod.Bass.to_json_bytes

        def to_json_bytes(self, *a, **kw):
            data = json.loads(orig(self, *a, **kw))
            _split_sync_waits(data)
            return json.dumps(data).encode()

        bass_mod.Bass.to_json_bytes = to_json_bytes
        bass_mod.Bass._waitsplit_patched = True
    if not getattr(tile_mod.TileContext, "_drain_patched", False):
        import bass_rust
        ScopedClock = tile_mod.ScopedClock

        def _drain_and_barrier(self, tick_clock, wait_clock):
            drain_inst = self.nc.sync.drain()
            wait_clock.add_sem_waits(
                drain_inst.ins, ScopedClock({None: tick_clock.global_clock})
            )
            si = drain_inst.ins.sync_info
            if si is not None and len(si.on_wait) > 1:
                waits = list(si.on_wait)
                drain_inst.ins.sync_info = bass_rust.SyncInfo(
                    on_wait=waits[:1], on_update=list(si.on_update)
                )
                for w in waits[1:]:
                    extra = self.nc.sync.drain()
                    extra.ins.sync_info = bass_rust.SyncInfo(on_wait=[w], on_update=[])
            self.nc.all_engine_barrier()
            assert self.sems is not None
            popped = self.nc._tile_sem_poison_stack.pop()
            assert popped is self._sem_poison
            self.nc.clear_and_free_semaphores(list(self.sems.allocated().values()))
            self.nc.all_engine_barrier()

        tile_mod.TileContext._drain_and_barrier = _drain_and_barrier
        tile_mod.TileContext._drain_patched = True

    try:
        import antenv
        if "antenv.axon_hooks" not in sys.modules:
            mod = types.ModuleType("antenv.axon_hooks")
            mod._hook = None

            def set_axon_ntff_profile_hook(h, _mod=mod):
                _mod._hook = h

            def get_axon_ntff_profile_hook(_mod=mod):
                return _mod._hook

            mod.set_axon_ntff_profile_hook = set_axon_ntff_profile_hook
            mod.get_axon_ntff_profile_hook = get_axon_ntff_profile_hook
            sys.modules["antenv.axon_hooks"] = mod
            antenv.axon_hooks = mod
        from trn_agent_boot.trn_boot import _ntff_profile_via_ctypes
        hook = _ntff_profile_via_ctypes("/opt/axon/libaxon_pjrt.so")
        sys.modules["antenv.axon_hooks"].set_axon_ntff_profile_hook(hook)
        import concourse.bass_utils as bass_utils
        bass_utils.upload_artifacts = lambda d: d
    except Exception:
        pass


def _build_nc():
    import concourse.bass as bass
    import concourse.tile as tile
    from concourse import mybir

    f32 = mybir.dt.float32
    bf16 = mybir.dt.bfloat16
    AF = mybir.ActivationFunctionType
    MULT = mybir.AluOpType.mult
    ADD = mybir.AluOpType.add
    BYPASS = mybir.AluOpType.bypass

    nc = bass.Bass("TRN2", target_bir_lowering=False, debug=False,
                   num_devices=NCORES)
    xT_d = nc.dram_tensor("xT", [D, S], bf16, kind="ExternalInput").ap()
    wqk_d = nc.dram_tensor("wqk", [D, 512], bf16, kind="ExternalInput").ap()
    bqk_d = nc.dram_tensor("bqk", [128, 4], f32, kind="ExternalInput").ap()
    wv_d = nc.dram_tensor("wv", [D, VROW], bf16, kind="ExternalInput").ap()
    bv_d = nc.dram_tensor("bv", [128, VROW], f32, kind="ExternalInput").ap()
    wo_d = nc.dram_tensor("wo", [256, D], bf16, kind="ExternalInput").ap()
    out_d = nc.dram_tensor("out", [D, S], f32, kind="ExternalOutput").ap()

    def zview(zt):
        # bf16 exp values = low halfword of each magic f32
        return zt[:].bitcast(bf16).rearrange("p (n two) -> p n two", two=2)[:, :, 0]

    with tile.TileContext(nc) as tc, ExitStack() as ctx:
        persist = ctx.enter_context(tc.tile_pool(name="persist", bufs=1))
        # PSUM budget (8 banks): ps 3x[128,1024]=6 banks, pvp [65,1024]=2.
        ps = ctx.enter_context(
            tc.tile_pool(name="ps", bufs=3, space=bass.MemorySpace.PSUM))
        pvp = ctx.enter_context(
            tc.tile_pool(name="pvp", bufs=1, space=bass.MemorySpace.PSUM))
        expp = ctx.enter_context(tc.tile_pool(name="expp", bufs=3))
        zp = ctx.enter_context(tc.tile_pool(name="zp", bufs=3))
        lnp = ctx.enter_context(tc.tile_pool(name="lnp", bufs=2))
        rec64p = ctx.enter_context(tc.tile_pool(name="rec64p", bufs=2))
        outp = ctx.enter_context(tc.tile_pool(name="outp", bufs=2))

        xT_sb = persist.tile([128, 8 * S], bf16)
        wqk_sb = persist.tile([128, 8 * 512], bf16)
        bqk_sb = persist.tile([128, 4], f32)
        wv_sb = persist.tile([128, 8 * VROW], bf16)
        bv_sb = persist.tile([128, VROW], f32)
        wo_sb = persist.tile([128, 2 * D], bf16)
        qkT_sb = persist.tile([128, 4 * S], bf16)
        v_sb = persist.tile([128, NKB * VROW], bf16)
        outT_sb = persist.tile([128, 2 * S], bf16)
        ones64 = persist.tile([1, 64], f32)
        nc.vector.memset(ones64[:], 1.0)

        for kb in range(8):
            nc.sync.dma_start(xT_sb[:, kb * S:(kb + 1) * S],
                              xT_d[kb * 128:(kb + 1) * 128, :])
            nc.sync.dma_start(wqk_sb[:, kb * 512:(kb + 1) * 512],
                              wqk_d[kb * 128:(kb + 1) * 128, :])
            nc.sync.dma_start(wv_sb[:, kb * VROW:(kb + 1) * VROW],
                              wv_d[kb * 128:(kb + 1) * 128, :])
        nc.sync.dma_start(bqk_sb[:], bqk_d[:])
        nc.sync.dma_start(bv_sb[:], bv_d[:])
        for cb in range(2):
            nc.sync.dma_start(wo_sb[:, cb * D:(cb + 1) * D],
                              wo_d[cb * 128:(cb + 1) * 128, :])

        # qkT projection: m-tiles m0=[q_h0|q_h1] m1=[q_h2|q_h3] m2=[k_h0|k_h1]
        # m3=[k_h2|k_h3]; bias-add on DVE (per-partition scalar).
        for m in range(4):
            for npair in range(2):
                ss = ps.tile([128, 2 * QT], f32, tag="s")
                for kb in range(8):
                    for half in range(2):
                        n = npair * 2 + half
                        nc.tensor.matmul(
                            ss[:, half * QT:(half + 1) * QT],
                            wqk_sb[:, kb * 512 + m * 128: kb * 512 + (m + 1) * 128],
                            xT_sb[:, kb * S + n * QT: kb * S + (n + 1) * QT],
                            start=(kb == 0), stop=(kb == 7),
                            skip_group_check=True)
                nc.vector.tensor_scalar(
                    qkT_sb[:, m * S + npair * 2 * QT: m * S + (npair + 1) * 2 * QT],
                    ss[:], bqk_sb[:, m:m + 1], None, ADD, BYPASS)

        # v projection (+ ones column via bv): v_sb[128, 16*260]
        for sb in range(NKB):
            ss = ps.tile([128, 2 * QT], f32, tag="s")
            for kb in range(8):
                nc.tensor.matmul(
                    ss[:, 0:VROW],
                    xT_sb[:, kb * S + sb * 128: kb * S + (sb + 1) * 128],
                    wv_sb[:, kb * VROW:(kb + 1) * VROW],
                    start=(kb == 0), stop=(kb == 7))
            nc.vector.tensor_add(
                v_sb[:, sb * VROW:(sb + 1) * VROW], ss[:, 0:VROW], bv_sb[:])

        # attention per (query pair qp, head pair hp, query half qh).
        # Each kb produces ONE psum tile holding [head-A | head-B]
        # scores for this query half: the two matmuls into one tile
        # stay adjacent in the PE stream and their disjoint row groups
        # (A rows 0-63, B rows 64-127) run concurrently. The single
        # scores stream is triple-buffered (ps bufs=3) so the PE never
        # waits on the exp engines; exp alternates ACT/DVE by kb.
        def make_finisher(pv2, hp, qp, qh):
            # pv2 [65, 2*QT] = [A | B] for query half qh; row 64 holds
            # the softmax denominators. Deferred past the next
            # iteration's first score tiles. The two ACT reads (Ln +
            # copy to SBUF) release pv2 early; the rest of the
            # reciprocal chain works off the copy, off the PE critical
            # path.
            def emit():
                ocol = hp * S + qp * 2 * QT + qh * QT
                lnrow = lnp.tile([1, 2 * QT], f32, tag="ln")
                nc.scalar.activation(lnrow[:], pv2[64:65, :], AF.Ln)
                ps_rec = ps.tile([128, 2 * QT], f32, tag="s")
                for j in range(2):
                    pr = j * 64
                    nc.tensor.matmul(
                        ps_rec[pr:pr + 64, 0:QT],
                        ones64[:], lnrow[:, j * QT:(j + 1) * QT],
                        start=True, stop=True,
                        tile_position=(0, pr), skip_group_check=True)
                rec64 = rec64p.tile([128, QT], f32, tag="r64")
                nc.scalar.activation(rec64[:], ps_rec[:, 0:QT], AF.Exp,
                                     scale=-1.0)
                for j in range(2):
                    pr = j * 64
                    nc.vector.tensor_tensor(
                        out=outT_sb[pr:pr + 64, ocol: ocol + QT],
                        in0=pv2[0:64, j * QT:(j + 1) * QT],
                        in1=rec64[pr:pr + 64, :],
                        op=MULT)
            return emit

        pending = None
        for qp in range(2):
            for hp in range(2):
                kbase = (2 + hp) * S
                hA, hB = 2 * hp, 2 * hp + 1
                for qh in range(2):
                    qs = hp * S + qp * 2 * QT + qh * QT
                    pv2 = pvp.tile([VW, 2 * QT], f32, tag="pv2", name="pv2")
                    lag = []  # PV emission lags 2 kb behind scores/exp
                    for kb in range(NKB):
                        kA = qkT_sb[0:64,
                                    kbase + kb * 128: kbase + (kb + 1) * 128]
                        kB = qkT_sb[64:128,
                                    kbase + kb * 128: kbase + (kb + 1) * 128]
                        s = ps.tile([128, 2 * QT], f32, tag="s", name="s")
                        nc.tensor.matmul(
                            s[:, 0:QT], kA, qkT_sb[0:64, qs: qs + QT],
                            start=True, stop=True, skip_group_check=True)
                        nc.tensor.matmul(
                            s[:, QT:2 * QT], kB, qkT_sb[64:128, qs: qs + QT],
                            start=True, stop=True, skip_group_check=True)
                        if kb % 2 == 0:
                            e0 = expp.tile([128, 2 * QT], bf16, tag="e")
                            nc.scalar.activation(e0[:], s[:], AF.Exp)
                            rhs = e0[:]
                        else:
                            z1 = zp.tile([128, 2 * QT], f32, tag="z")
                            nc.vector.tensor_scalar(z1[:], s[:],
                                                    EXP_A, EXP_B, MULT, ADD)
                            rhs = zview(z1)
                        if pending is not None:
                            pending()
                            pending = None
                        lag.append((rhs, kb))
                        if len(lag) > 2:
                            emit_pv(nc, pv2, v_sb, hA, hB, *lag.pop(0))
                    emit_pv(nc, pv2, v_sb, hA, hB, *lag.pop(0))
                    emit_pv(nc, pv2, v_sb, hA, hB, *lag.pop(0), last=True)
                    pending = make_finisher(pv2, hp, qp, qh)

            pending()
            pending = None

            # out projection for this query pair (both tiles at once)
            for dt in range(8):
                ss = ps.tile([128, 2 * QT], f32, tag="s")
                for half in range(2):
                    qi = qp * 2 + half
                    for cb in range(2):
                        nc.tensor.matmul(
                            ss[:, half * QT:(half + 1) * QT],
                            wo_sb[:, cb * D + dt * 128: cb * D + (dt + 1) * 128],
                            outT_sb[:, cb * S + qi * QT: cb * S + (qi + 1) * QT],
                            start=(cb == 0), stop=(cb == 1),
                            skip_group_check=True)
                osb = outp.tile([128, 2 * QT], f32, tag="ob")
                if dt % 2 == 0:
                    nc.scalar.activation(osb[:], ss[:], AF.Identity)
                else:
                    nc.vector.tensor_copy(osb[:], ss[:])
                nc.sync.dma_start(
                    out_d[dt * 128:(dt + 1) * 128, qp * 2 * QT:(qp + 1) * 2 * QT],
                    osb[:])
    return nc


def emit_pv(nc, pv2, v_sb, hA, hB, rhs, kb, last=False):
    # pv2 layout: [A | B], each QT wide; rhs = exp tile [128, 2*QT]
    # with head A in the left half and head B in the right half.
    vA = v_sb[:, kb * VROW + hA * VW: kb * VROW + (hA + 1) * VW]
    vB = v_sb[:, kb * VROW + hB * VW: kb * VROW + (hB + 1) * VW]
    start = (kb == 0)
    stop = last
    nc.tensor.matmul(pv2[:, 0:QT], vA, rhs[:, 0:QT],
                     start=start, stop=stop, skip_group_check=True)
    nc.tensor.matmul(pv2[:, QT:2 * QT], vB, rhs[:, QT:2 * QT],
                     start=start, stop=stop, skip_group_check=True)


def _get_nc():
    if "nc" not in _CACHE:
        _install_support()
        _CACHE["nc"] = _build_nc()
    return _CACHE["nc"]


LAST_EXEC_NS = None


def kernel(x, Wqkv, bqkv, Wout, bout):
    from ml_dtypes import bfloat16
    from concourse.bass_utils import run_bass_kernel_spmd

    nc = _get_nc()

    x = np.asarray(x, np.float32)
    Wqkv = np.asarray(Wqkv, np.float32)
    bqkv = np.asarray(bqkv, np.float32)
    Wout = np.asarray(Wout, np.float32)
    bout = np.asarray(bout, np.float32)

    xT = [np.ascontiguousarray(x[b].T).astype(bfloat16) for b in range(B)]

    in_maps = []
    for c in range(NCORES):
        b, hg = divmod(c, HPC)
        heads = [hg * HPC + j for j in range(HPC)]

        wqk = np.empty((D, 512), np.float32)
        bqk = np.empty(512, np.float32)
        for j, h in enumerate(heads):
            base = h * 192
            wqk[:, j * 64:(j + 1) * 64] = Wqkv[:, base:base + 64] * 0.125
            wqk[:, 256 + j * 64: 256 + (j + 1) * 64] = Wqkv[:, base + 64:base + 128]
            bqk[j * 64:(j + 1) * 64] = bqkv[base:base + 64] * 0.125
            bqk[256 + j * 64: 256 + (j + 1) * 64] = bqkv[base + 64:base + 128]
        bqk = np.ascontiguousarray(bqk.reshape(4, 128).T)

        wv = np.zeros((D, VROW), np.float32)
        bv_row = np.zeros(VROW, np.float32)
        for j, h in enumerate(heads):
            base = h * 192 + 128
            wv[:, j * VW: j * VW + 64] = Wqkv[:, base:base + 64]
            bv_row[j * VW: j * VW + 64] = bqkv[base:base + 64]
            bv_row[j * VW + 64] = 1.0
        bv = np.broadcast_to(bv_row, (128, VROW)).copy()

        wo = np.ascontiguousarray(Wout[hg * 256:(hg + 1) * 256, :])

        in_maps.append({
            "xT": xT[b],
            "wqk": wqk.astype(bfloat16),
            "bqk": bqk,
            "wv": wv.astype(bfloat16),
            "bv": bv,
            "wo": wo.astype(bfloat16),
        })

    res = run_bass_kernel_spmd(nc, in_maps, core_ids=list(range(NCORES)))
    global LAST_EXEC_NS
    LAST_EXEC_NS = getattr(res, "exec_time_ns", None)

    out = np.empty((B, S, D), np.float32)
    for b in range(B):
        acc = res.results[b * HPC + 0]["out"].astype(np.float32)
        for hg in range(1, HPC):
            acc = acc + res.results[b * HPC + hg]["out"].astype(np.float32)
        out[b] = acc.T + bout
    return out
